# revision 40
# baseline (speedup 1.0000x reference)
"""DGI (2-layer GCN encoder x2 + bilinear discriminator) on 8 Trainium2 cores.

Strategy (v2: flipped scatter)
-----------------------------
Both encodes share the graph, so they are fused into one 128-wide feature
matrix ([x-encode 64 | cfeat-encode 64]).  The symmetric GCN normalization is
factored as  A_hat @ H = diag(dinv) @ A01 @ (diag(dinv) @ H); the src-side
dinv is pre-folded into the gathered table, and the dst-side dinv is deferred
out of the SpMM entirely (relu is positive-homogeneous, so layer-2's table
picks up dinv^2 and the discriminator picks up the final dinv -- both as
exact per-partition f32 scales).

SpMM: messages are dst-sorted, so 128 consecutive messages span only ~25-40
destination columns.  Each message tile is loaded once as the stationary
matmul operand ([slot x feat]) and multiplied by a narrow 0/1 one-hot
([slot x 64]) into a PSUM accumulator of [128 feat x 896 dst]; self loops are
identity matmuls of the core's own table block.  This costs ~1.5 cyc/message
on the tensor engine vs ~2.7 for the dst-major formulation, and shrinks the
DVE one-hot build by 2x.

Sharding: nodes are split into 8 contiguous ranges (12500/core, padded to
12544).  Each core computes its rows of the gather table (dense matmul),
AllGathers the full bf16 table, then processes edges whose dst lands in its
range.  Source rows are fetched per-edge with SWDGE dma_gather; calls rotate
across the 4 SWDGE queues so descriptor generation runs on all four Q7 core
pairs concurrently.  Src chunks of <=32512 padded rows keep gather indices
within int16.

Discriminator reduces to  sc = sigmoid(mean(h1)) * (h @ rowsum(Wd)) + bd,
computed per dst block as one [feat x 4] matmul against the layer-2 window.
"""

import numpy as np
import ml_dtypes

import concourse.bass as bass
import concourse.bacc as bacc
import concourse.mybir as mybir
import concourse.tile as tile
from concourse import bass_utils
from concourse.library_config import mlp

BF16 = ml_dtypes.bfloat16

N = 100000
E = 1600000
IN_D = 128
HID = 64
OUT_D = 64
C = 8                 # cores
S = N // C            # 12500 nodes per core
B = 98                # dst blocks of 128 per core (98*128 = 12544)
SP = B * 128          # padded shard rows
P = 128
WCOLS = 896           # psum window dst columns (7 blocks)
NW = (B * P) // WCOLS  # 14 windows
GB = WCOLS // P       # 7 blocks per window
BIN = 64              # one-hot width / dst-column bin (divides the psum bank)
NBIN = WCOLS // BIN   # 14 bins per window
# src chunks = table quarters, window-aligned so each quarter's AllGather can
# fire as soon as its windows of the producing phase are done.  Chunk row
# counts (x8 cores) stay within the int16 gather-index limit.
NCH = 4
QW = [0, 4, 8, 11, 14]                 # window boundaries per quarter
LQ = [0, 3584, 7168, 9856, 12544]      # local row boundaries per quarter
QR = [3584, 3584, 2688, 2688]          # rows per core per quarter


def _preprocess(edge_index):
    """Sort edges into the dst-sorted per-core streamed tile layout.

    Returns per-core idx/dloc arrays plus the (shared) tile/matmul schedule.
    """
    ei = np.asarray(edge_index).astype(np.int64)
    src = ei[0]
    dst = ei[1]
    # degree includes the self loop; the self-loop message itself is not
    # gathered -- it is added on-device as an identity matmul of the core's
    # own table rows.
    deg = (np.bincount(dst, minlength=N) + 1).astype(np.float32)
    dinv = (1.0 / np.sqrt(deg)).astype(np.float32)

    core = dst // S
    dl = (dst % S).astype(np.int64)          # dst column within core
    win = dl // WCOLS                        # psum window
    slocal = src % S
    chk = np.searchsorted(np.asarray(LQ), slocal, side="right") - 1
    qr = np.asarray(QR)
    lq = np.asarray(LQ)
    # row within the quarter-q AllGathered table [C*QR[q], 128]
    sloc = ((src // S) * qr[chk] + slocal - lq[chk]).astype(np.int32)

    nseg = NW * NCH
    segkey = (core * NW + win) * NCH + chk
    order = np.lexsort((sloc, dl, segkey))
    seg_s = segkey[order]
    dl_s = dl[order]
    sl_s = sloc[order]

    cnt = np.bincount(segkey, minlength=C * nseg).reshape(C, NW, NCH)
    # shared tile count per (window, chunk): cross-core max
    T_wc = -(-cnt.max(axis=0) // P)          # [NW, NCH]
    toff = np.zeros((NW, NCH), np.int64)     # tile offset within window
    for w in range(NW):
        t = 0
        for c in range(NCH):
            toff[w, c] = t
            t += T_wc[w, c]
    T_w = T_wc.sum(axis=1)                   # tiles per window
    wbase = np.zeros(NW + 1, np.int64)
    np.cumsum(T_w, out=wbase[1:])
    TOTT = int(wbase[NW])

    # per-msg stream position (global tile, slot)
    starts = np.zeros(C * nseg + 1, np.int64)
    np.cumsum(np.bincount(seg_s, minlength=C * nseg), out=starts[1:])
    rank = np.arange(seg_s.size) - starts[seg_s]
    w_s = (seg_s // NCH) % NW
    c_s = seg_s % NCH
    gt = wbase[w_s] + toff[w_s, c_s] + rank // P      # global stream tile
    slot = rank % P

    # per (w, c, t): union bin range across cores -> matmul entries
    Tmax = int(T_wc.max())
    blo = np.full((NW, NCH, Tmax), NBIN, np.int64)
    bhi = np.full((NW, NCH, Tmax), -1, np.int64)
    for r in range(C):
        for w in range(NW):
            for c in range(NCH):
                n = int(cnt[r, w, c])
                if n == 0:
                    continue
                s0 = int(starts[(r * NW + w) * NCH + c])
                d = dl_s[s0:s0 + n] - w * WCOLS
                for t in range(-(-n // P)):
                    lo = int(d[t * P]) // BIN
                    hi = int(d[min((t + 1) * P, n) - 1]) // BIN
                    blo[w, c, t] = min(blo[w, c, t], lo)
                    bhi[w, c, t] = max(bhi[w, c, t], hi)

    # entry list per window: (tile index within window, bin).  mms merges
    # adjacent-bin entries of the same tile into one wider matmul when the
    # merged region stays within one 512-col psum bank.
    schedule = []                     # [NW] list of (t_local, bin)
    mms = []                          # [NW] list of (t_local, bin, k, nbins)
    eid = np.full((NW, NCH, Tmax, NBIN), -1, np.int64)
    k = 0
    for w in range(NW):
        ents = []
        for c in range(NCH):
            for t in range(int(T_wc[w, c])):
                for b in range(int(blo[w, c, t]), int(bhi[w, c, t]) + 1):
                    eid[w, c, t, b] = k
                    ents.append((int(toff[w, c] + t), b))
                    k += 1
        schedule.append(ents)
        wm = []
        j = 0
        k0 = k - len(ents)
        while j < len(ents):
            tl, b = ents[j]
            nb = 1
            while (j + nb < len(ents) and ents[j + nb] == (tl, b + nb)
                   and (b * BIN) // 512 == ((b + nb + 1) * BIN - 1) // 512):
                nb += 1
            wm.append((tl, b, k0 + j, nb))
            j += nb
        mms.append(wm)
    TOTK = k

    # per-msg entry id and local one-hot column
    t_loc = rank // P
    bin_s = (dl_s - w_s * WCOLS) // BIN
    ek = eid[w_s, c_s, t_loc, bin_s]
    assert (ek >= 0).all()
    dcol = dl_s - w_s * WCOLS - bin_s * BIN   # 0..BIN-1

    idx_cores = []
    dloc_cores = []
    core_s = seg_s // (NW * NCH)
    for r in range(C):
        msk = core_s == r
        # pad slots gather row 0 (valid data, killed by the 255 one-hot);
        # negative-index stripping desyncs the SWDGE ring bookkeeping
        # (decode reserves from num_idxs_reg, ucode generates post-strip).
        SRC = np.zeros(TOTT * P, np.int16)
        SRC[(gt[msk] * P + slot[msk])] = sl_s[msk].astype(np.int16)
        # idx packing for dma_gather: call-local index i -> [i%16, i//16],
        # replicated across the 8 groups of 16 partitions.  Calls are tile
        # aligned so packing the whole stream at once keeps every call's
        # columns self-contained.
        a = SRC.reshape(-1, 16).T                      # [16, TOTT*8]
        idx_cores.append(np.tile(a, (8, 1)).copy())    # [128, TOTT*8]

        DLC = np.full((TOTK, P), 255, np.int64)
        DLC[ek[msk], slot[msk]] = dcol[msk]
        dloc_cores.append(
            np.ascontiguousarray(DLC.T).astype(BF16))  # [128, TOTK]

    return dict(
        dinv=dinv,
        schedule=schedule,
        mms=mms,
        T_wc=T_wc,
        toff=toff,
        T_w=T_w,
        wbase=wbase,
        TOTT=TOTT,
        TOTK=TOTK,
        idx_cores=idx_cores,
        dloc_cores=dloc_cores,
    )


def _build(pp, stage=5, debug=False):
    """Build the 8-core SPMD bass program."""
    schedule = pp["schedule"]
    mms = pp["mms"]
    T_wc = pp["T_wc"]
    toff = pp["toff"]
    T_w = pp["T_w"]
    wbase = pp["wbase"]
    TOTT = pp["TOTT"]
    TOTK = pp["TOTK"]
    WTmax = int(T_w.max())
    WKmax = max(len(e) for e in schedule)

    nc = bacc.Bacc("TRN2", target_bir_lowering=False, debug=False, num_devices=C,
                   num_swdge_queues=4)
    f32 = mybir.dt.float32
    bf16 = mybir.dt.bfloat16
    i16 = mybir.dt.int16

    t_xs = nc.dram_tensor("xs", [SP, P], bf16, kind="ExternalInput")
    t_cs = nc.dram_tensor("cs", [SP, P], bf16, kind="ExternalInput")
    t_w1 = nc.dram_tensor("w1", [P, HID], bf16, kind="ExternalInput")
    t_w2d = nc.dram_tensor("w2d", [P, P], bf16, kind="ExternalInput")
    t_discw = nc.dram_tensor("discw", [P, 4], bf16, kind="ExternalInput")
    t_iota = nc.dram_tensor("iota64", [P, BIN], bf16, kind="ExternalInput")
    t_dinv = nc.dram_tensor("dinvc", [P, B], f32, kind="ExternalInput")
    t_dinv2 = nc.dram_tensor("dinv2c", [P, B], f32, kind="ExternalInput")
    t_idx = nc.dram_tensor("idx", [P, TOTT * 8], i16, kind="ExternalInput")
    t_dloc = nc.dram_tensor("dloc", [P, TOTK], bf16, kind="ExternalInput")
    t_out = nc.dram_tensor("out", [2, B, P], f32, kind="ExternalOutput")
    if debug:
        t_dbgh = nc.dram_tensor("dbgh", [P, B * P], bf16, kind="ExternalOutput")
        t_dbgo = nc.dram_tensor("dbgo", [P, B * P], bf16, kind="ExternalOutput")
        t_dbgg = nc.dram_tensor("dbgg", [P, 4 * B], f32, kind="ExternalOutput")
        t_dbgm = nc.dram_tensor("dbgm", [2, P, 128 * P], bf16,
                                kind="ExternalOutput")

    t1_sh = nc.dram_tensor("t1sh", [SP, P], bf16, kind="Internal")
    t2_sh = nc.dram_tensor("t2sh", [SP, P], bf16, kind="Internal")
    t1_fq = [nc.dram_tensor(f"t1f{q}", [C * QR[q], P], bf16, kind="Internal",
                            addr_space="Shared") for q in range(NCH)]
    t2_fq = [nc.dram_tensor(f"t2f{q}", [C * QR[q], P], bf16, kind="Internal",
                            addr_space="Shared") for q in range(NCH)]

    Copy = mybir.ActivationFunctionType.Copy
    Relu = mybir.ActivationFunctionType.Relu
    Sigmoid = mybir.ActivationFunctionType.Sigmoid

    with tile.TileContext(nc) as tc:
        nc.gpsimd.load_library(mlp)
        with (
            tc.tile_pool(name="const", bufs=1) as constp,
            tc.tile_pool(name="hbuf", bufs=1) as hbufp,
            tc.tile_pool(name="io", bufs=3) as iop,
            tc.tile_pool(name="ioA", bufs=2) as iopA,
            tc.tile_pool(name="msgs", bufs=3) as msgp,
            tc.tile_pool(name="oh", bufs=1) as ohp,
            tc.tile_pool(name="y2", bufs=2) as y2p,
            tc.tile_pool(name="psA", bufs=2, space="PSUM") as psA,
            tc.tile_pool(name="psW", bufs=2, space="PSUM") as psW,
            tc.tile_pool(name="psT", bufs=1, space="PSUM") as psT,
            tc.tile_pool(name="small", bufs=4) as smallp,
        ):
            # ---- constants ----
            w1_sb = constp.tile([P, HID], bf16)
            nc.sync.dma_start(w1_sb[:], t_w1.ap())
            w2d_sb = constp.tile([P, P], bf16)
            nc.sync.dma_start(w2d_sb[:], t_w2d.ap())
            discw_sb = constp.tile([P, 4], bf16)
            nc.sync.dma_start(discw_sb[:], t_discw.ap())
            iota_sb = constp.tile([P, BIN], bf16)
            nc.sync.dma_start(iota_sb[:], t_iota.ap())
            dinv_sb = constp.tile([P, B], f32)
            nc.sync.dma_start(dinv_sb[:], t_dinv.ap())
            dinv2_sb = constp.tile([P, B], f32)
            nc.sync.dma_start(dinv2_sb[:], t_dinv2.ap())
            ident_sb = constp.tile([P, P], f32)
            from concourse.masks import make_identity
            make_identity(nc, ident_sb[:])
            ident_bf = constp.tile([P, P], bf16)
            nc.vector.tensor_copy(ident_bf[:], ident_sb[:])

            own_buf = hbufp.tile([P, B * P], bf16)   # this core's table rows
            h_fm = hbufp.tile([P, B * P], bf16)      # layer-1 out, [feat x dst]
            grid = hbufp.tile([P, 4 * B], f32)       # disc per-block results

            # AllGathers are fired lazily (idempotent): each AG instruction
            # WAITS for its input DMA on the gpsimd stream, so firing early
            # stalls all later gather desc-gen queued behind it.
            ag_done = set()

            def fire_ag(lay, q):
                if (lay, q) in ag_done:
                    return
                ag_done.add((lay, q))
                sh, fq = (t1_sh, t1_fq) if lay == 1 else (t2_sh, t2_fq)
                nc.gpsimd.collective_compute(
                    "AllGather", mybir.AluOpType.bypass,
                    replica_groups=[list(range(C))],
                    ins=[sh.ap()[LQ[q]:LQ[q + 1], :]],
                    outs=[fq[q].ap()],
                )

            # ---- phase A: T1 = dinv * [x@W1 | c@W1]  (bf16 table) ----
            # 14 node-tiles per DMA (transpose loads) to amortize the ~1us
            # per-DMA fixed cost; each table quarter AllGathers as soon as its
            # blocks are written so layer-1 gathers start early.
            GA = 14
            for q in range(NCH):
                b0, b1 = QW[q] * GB, QW[q + 1] * GB
                for g0 in range(b0, b1, GA):
                    ng = min(GA, b1 - g0)
                    xt = iopA.tile([P, GA * P], bf16, tag="xt")
                    nc.sync.dma_start(xt[:, :ng * P],
                                      t_xs.ap()[g0 * P:(g0 + ng) * P, :],
                                      transpose=True)
                    ct = iopA.tile([P, GA * P], bf16, tag="ct")
                    nc.sync.dma_start(ct[:, :ng * P],
                                      t_cs.ap()[g0 * P:(g0 + ng) * P, :],
                                      transpose=True)
                    for j in range(ng):
                        i = g0 + j
                        ps = psA.tile([P, P], f32, tag="psd")
                        nc.tensor.matmul(ps[:, :HID], xt[:, j * P:(j + 1) * P],
                                         w1_sb[:], start=True, stop=True)
                        nc.tensor.matmul(ps[:, HID:], ct[:, j * P:(j + 1) * P],
                                         w1_sb[:], start=True, stop=True)
                        nc.scalar.activation(own_buf[:, i * P:(i + 1) * P],
                                             ps[:], Copy,
                                             scale=dinv_sb[:, i:i + 1])
                    nc.sync.dma_start(
                        t1_sh.ap()[g0 * P:(g0 + ng) * P, :]
                            .rearrange("(b p) f -> p b f", p=P),
                        own_buf[:, g0 * P:(g0 + ng) * P]
                            .rearrange("p (b f) -> p b f", f=P))
            # table-1 AllGathers fire lazily from the spmm chunk loop: the
            # first gathers' desc-gen then overlaps later phase-A quarters.

            # ---- SpMM pass (shared for both layers) ----
            def spmm(tabq, layer):
                k0 = 0
                for w in range(NW):
                    if layer == 1:
                        # fire table-2 AllGathers two windows after their
                        # quarter completes: the t2_sh DMA has landed by then,
                        # so the collective's input wait is free.
                        for q in range(NCH):
                            if QW[q + 1] + 1 <= w:
                                fire_ag(2, q)
                    wt = int(T_w[w])
                    t0 = int(wbase[w])
                    ents = schedule[w]
                    wk = len(ents)
                    idxw = iop.tile([P, WTmax * 8], i16, tag="idxw")
                    nc.sync.dma_start(idxw[:, :wt * 8],
                                      t_idx.ap()[:, t0 * 8:(t0 + wt) * 8])
                    dlocw = iop.tile([P, WKmax], bf16, tag="dlocw")
                    nc.sync.dma_start(dlocw[:, :wk],
                                      t_dloc.ap()[:, k0:k0 + wk])
                    k0 += wk
                    msgs = msgp.tile([P, WTmax * P], bf16, tag="msgs")
                    if layer == 1 and w < 2:
                        # stripped (-1) gather pads leave slots unwritten;
                        # clear potential NaN garbage in fresh SBUF once.
                        nc.vector.memset(msgs[:], 0.0)
                    for c in range(NCH):
                        fire_ag(layer, c)
                        cnt_t = int(T_wc[w, c])
                        coff = int(toff[w, c])
                        # SWDGE ring limit: keep calls <= 44 tiles
                        for s0 in range(0, cnt_t, 44):
                            st = min(44, cnt_t - s0)
                            o = coff + s0
                            nidx = st * P
                            nc.gpsimd.dma_gather(
                                msgs[:, o * P:(o + st) * P].rearrange(
                                    "p (t d) -> p t d", d=P),
                                tabq[c].ap(),
                                idxw[:, o * 8:(o + st) * 8],
                                nidx, nidx, P, single_packet=False,
                                queue_num=(w + c) % 4,
                            )
                    if debug and layer == 1 and w in (0, 7):
                        nc.sync.dma_start(
                            t_dbgm.ap()[0 if w == 0 else 1][:, :wt * P],
                            msgs[:, :wt * P])
                    # one-hot build: ohg[:, k, j] = (dlocw[:, k] == j)
                    ohg = ohp.tile([P, WKmax * BIN], bf16, tag="ohg")
                    nc.vector.tensor_tensor(
                        out=ohg[:, :wk * BIN].rearrange(
                            "p (k d) -> p k d", d=BIN),
                        in0=dlocw[:, :wk].to_broadcast([P, wk, BIN]),
                        in1=iota_sb[:].rearrange("p (a d) -> p a d", a=1)
                            .to_broadcast([P, wk, BIN]),
                        op=mybir.AluOpType.is_equal)
                    psw = psW.tile([P, WCOLS], f32, tag="psw")
                    # self loops init the accumulators: psum[:, d] = T_own[d].
                    # start=True clears has_written for the WHOLE 512-col psum
                    # bank, so only the first matmul per bank may set it.
                    for b in range(GB):
                        gb = w * GB + b
                        nc.tensor.matmul(
                            psw[:, b * P:(b + 1) * P],
                            own_buf[:, gb * P:(gb + 1) * P], ident_bf[:],
                            start=(b % 4 == 0), stop=False,
                            skip_group_check=True)
                    wm = mms[w]
                    kw0 = k0 - wk   # ohg slot base of this window
                    for j, (tl, bn, ke, nb) in enumerate(wm):
                        kl = ke - kw0
                        nc.tensor.matmul(
                            psw[:, bn * BIN:(bn + nb) * BIN],
                            msgs[:, tl * P:(tl + 1) * P],
                            ohg[:, kl * BIN:(kl + nb) * BIN],
                            start=False, stop=(j == len(wm) - 1),
                            skip_group_check=True)
                    if layer == 1:
                        nc.scalar.activation(
                            h_fm[:, w * WCOLS:(w + 1) * WCOLS], psw[:], Relu)
                        # phase C for this window's blocks:
                        # T2 = dinv^2 * (relu(S1) @ blockdiag(W2, W2))
                        for b in range(GB):
                            gb = w * GB + b
                            ps = psA.tile([P, P], f32, tag="psd")
                            nc.tensor.matmul(ps[:],
                                             h_fm[:, gb * P:(gb + 1) * P],
                                             w2d_sb[:], start=True, stop=True)
                            nc.scalar.activation(
                                own_buf[:, gb * P:(gb + 1) * P], ps[:],
                                Copy, scale=dinv2_sb[:, gb:gb + 1])
                        nc.sync.dma_start(
                            t2_sh.ap()[w * WCOLS:(w + 1) * WCOLS, :]
                                .rearrange("(b p) f -> p b f", p=P),
                            own_buf[:, w * WCOLS:(w + 1) * WCOLS]
                                .rearrange("p (b f) -> p b f", f=P))
                    else:
                        # discriminator inputs, per dst block:
                        # [mean64(h1) | h1@w | h2@w | 0] before dinv scaling
                        y2w = y2p.tile([P, WCOLS], bf16, tag="y2w")
                        nc.scalar.activation(y2w[:], psw[:], Copy)
                        for b in range(GB):
                            gb = w * GB + b
                            ps3 = psT.tile([P, 4], f32, tag="ps3")
                            nc.tensor.matmul(
                                ps3[:], y2w[:, b * P:(b + 1) * P],
                                discw_sb[:], start=True, stop=True)
                            nc.scalar.activation(
                                grid[:, gb * 4:(gb + 1) * 4], ps3[:],
                                Copy, scale=dinv_sb[:, gb:gb + 1])

            if stage >= 2:
                spmm(t1_fq, layer=1)
            if debug:
                nc.sync.dma_start(t_dbgh.ap(), h_fm[:])
                nc.sync.dma_start(t_dbgo.ap(), own_buf[:])

            if stage >= 4:
                spmm(t2_fq, layer=2)
            else:
                nc.vector.memset(grid[:], 0.0)
            if debug:
                nc.sync.dma_start(t_dbgg.ap(), grid[:])

            # ---- discriminator: sc1 = (h1@w) * sigmoid(mean(h1)) ----
            grid_r = grid[:].rearrange("p (b f) -> p b f", f=4)
            csig = smallp.tile([P, B], f32, tag="csig")
            nc.scalar.activation(
                csig[:].rearrange("p (b a) -> p b a", a=1),
                grid_r[:, :, 0:1], Sigmoid)
            sc1_st = smallp.tile([P, B], f32, tag="sc1")
            sc2_st = smallp.tile([P, B], f32, tag="sc2")
            nc.vector.tensor_tensor(
                out=sc1_st[:].rearrange("p (b a) -> p b a", a=1),
                in0=grid_r[:, :, 1:2],
                in1=csig[:].rearrange("p (b a) -> p b a", a=1),
                op=mybir.AluOpType.mult)
            nc.vector.tensor_tensor(
                out=sc2_st[:].rearrange("p (b a) -> p b a", a=1),
                in0=grid_r[:, :, 2:3],
                in1=csig[:].rearrange("p (b a) -> p b a", a=1),
                op=mybir.AluOpType.mult)

            for j, st in enumerate((sc1_st, sc2_st)):
                pso = psT.tile([B, P], f32, tag="pstr")
                nc.tensor.transpose(pso[:], st[:], ident_sb[:])
                so = smallp.tile([B, P], f32, tag="so")
                nc.scalar.activation(so[:], pso[:], Copy)
                nc.sync.dma_start(t_out.ap()[j], so[:])

    nc.compile()
    return nc


_CACHE = {}


def _run(inputs, trace=False):
    x = np.asarray(inputs["x"], np.float32)
    cfeat = np.asarray(inputs["cfeat"], np.float32)
    edge_index = inputs["edge_index"]
    W1 = np.asarray(inputs["W1"], np.float32)
    b1 = np.asarray(inputs["b1"], np.float32)
    W2 = np.asarray(inputs["W2"], np.float32)
    b2 = np.asarray(inputs["b2"], np.float32)
    Wd = np.asarray(inputs["Wd"], np.float32)
    bd = np.asarray(inputs["bd"], np.float32)
    assert not np.any(b1 != 0) and not np.any(b2 != 0), \
        "nonzero GCN biases unsupported (deferred-dinv formulation)"

    pp = _preprocess(edge_index)

    import os
    stage = int(os.environ.get("KERNEL_STAGE", "5"))
    debug = os.environ.get("KERNEL_DEBUG", "0") == "1"
    key = ("nc", int(np.asarray(edge_index)[0, 0]), pp["TOTT"], pp["TOTK"],
           stage, debug)
    if key not in _CACHE:
        _CACHE.clear()
        _CACHE[key] = _build(pp, stage, debug)
    nc = _CACHE[key]

    w = Wd.sum(axis=1).astype(np.float32)          # [64]
    discw = np.zeros((P, 4), np.float32)
    discw[:HID, 0] = 1.0 / HID
    discw[:HID, 1] = w
    discw[HID:, 2] = w
    discw = discw.astype(BF16)
    w1b = W1.astype(BF16)
    w2d = np.zeros((P, P), np.float32)
    w2d[:HID, :HID] = W2
    w2d[HID:, HID:] = W2
    w2d = w2d.astype(BF16)
    iota = np.tile(np.arange(BIN, dtype=np.float32)[None, :],
                   (P, 1)).astype(BF16)

    dinv = pp["dinv"]
    in_maps = []
    for r in range(C):
        xs = np.zeros((SP, P), np.float32)
        xs[:S] = x[r * S:(r + 1) * S]
        cs = np.zeros((SP, P), np.float32)
        cs[:S] = cfeat[r * S:(r + 1) * S]
        dv = np.ones(SP, np.float32)
        dv[:S] = dinv[r * S:(r + 1) * S]
        in_maps.append(dict(
            xs=xs.astype(BF16), cs=cs.astype(BF16),
            w1=w1b, w2d=w2d, discw=discw, iota64=iota,
            dinvc=np.ascontiguousarray(dv.reshape(B, P).T),
            dinv2c=np.ascontiguousarray((dv * dv).reshape(B, P).T),
            idx=pp["idx_cores"][r], dloc=pp["dloc_cores"][r],
        ))

    res = bass_utils.run_bass_kernel_spmd(
        nc, in_maps, core_ids=list(range(C)), trace=trace)

    sc1 = np.empty(N, np.float32)
    sc2 = np.empty(N, np.float32)
    for r in range(C):
        o = res.results[r]["out"].reshape(2, SP)
        sc1[r * S:(r + 1) * S] = o[0, :S]
        sc2[r * S:(r + 1) * S] = o[1, :S]
    logits = np.concatenate([sc1 + bd[0], sc2 + bd[0]])[None, :].astype(np.float32)
    return logits, res


def kernel(**inputs):
    logits, _ = _run(inputs, trace=False)
    return logits
